# revision 2
# baseline (speedup 1.0000x reference)
"""Inverse 2D wavelet block (3-level polyphase synthesis cascade) on 8 TRN2 cores.

Strategy
--------
Data-parallel over batch: core i processes batch element i (b=8, 8 cores).

Each level (N -> 2N) is two 1D synthesis passes expressed as TensorE matmuls
against host-precomputed banded synthesis matrices A (for h) and D (for g):

    out = (A @ ss + D @ sd) @ A^T  +  (A @ ds + D @ dd) @ D^T

Key trick: every matmul uses the *data* as the stationary operand (lhsT) and a
block of A^T/D^T as the moving operand, so the output of each pass lands in
exactly the layout the next pass needs as lhsT - the whole 3-level cascade runs
with zero transposes:

  step1:  sT[c, m]   = sum_j  ss[j, c] * AT[j, m]   (lhsT = ss block, as stored)
  step2:  out[r, mc] = sum_c  sT[c, r] * AT[c, mc]  (lhsT = sT block, as stored)

A^T is banded: j-block Jb (128 rows) only touches output columns
[256*Jb - 4, 256*Jb + 260).  So each (output-block, window) pair needs exactly
one 128-contraction matmul per input tensor, into a [128, <=264] PSUM window
(one bank).  Adjacent windows overlap by 8 columns; the overlap strip is summed
with a small DVE add during PSUM->SBUF evacuation.

The A^T blocks are translation invariant, so only 4 distinct [128, 2, 264]
constant blocks exist across all levels (left edge / interior / right edge /
single-window N=128), shipped as one small input tensor.
"""

import os
import numpy as np

import concourse.bass as bass
import concourse.mybir as mybir
import concourse.tile as tile
from concourse import bacc
from concourse.bass_utils import run_bass_kernel_spmd

F32 = mybir.dt.float32

K = 10
PAD = 2
T = K // 2  # 5 taps

# per-level 1D sizes, coarse->fine
LEVELS = ((2, 128), (1, 256), (0, 512))
IN_SHAPES = {
    'ss': 128, 'sd2': 128, 'ds2': 128, 'dd2': 128,
    'sd1': 256, 'ds1': 256, 'dd1': 256,
    'sd0': 512, 'ds0': 512, 'dd0': 512,
}
# windows per level: (lo, hi, const-block index)
WINS_BY_N = {
    128: [(0, 256, 3)],
    256: [(0, 260, 0), (252, 512, 2)],
    512: [(0, 260, 0), (252, 516, 1), (508, 772, 1), (764, 1024, 2)],
}
CST_W = 264  # padded width of constant blocks

# ---------------------------------------------------------------- host math

def _polyphase(f):
    return np.flip(f.reshape(-1, 2).T, axis=1)  # [2, T]


def _synth_AT(f, N):
    """A^T [N, 2N] with y[m] = sum_j a[j] * AT[j, m] (odd-reflect pad baked in)."""
    fk = _polyphase(f).astype(np.float64)
    I = np.eye(N)
    Ip = np.pad(I, [(0, 0), (PAD, PAD)], mode='reflect', reflect_type='odd')
    W = np.stack([Ip[:, t:t + N] for t in range(T)], axis=-1)  # [N, N, T]
    return np.einsum('jnt,pt->jnp', W, fk).reshape(N, 2 * N)


def _build_cst(h, g):
    """Pack the 4 distinct A^T/D^T blocks into [128, 4, 2, 264] f32."""
    cst = np.zeros((128, 4, 2, CST_W), np.float64)
    for i, f in enumerate((h, g)):
        AT512 = _synth_AT(f.astype(np.float64), 512)
        AT128 = _synth_AT(f.astype(np.float64), 128)
        cst[:, 0, i, :260] = AT512[0:128, 0:260]      # left edge
        cst[:, 1, i, :264] = AT512[128:256, 252:516]  # interior
        cst[:, 2, i, :260] = AT512[384:512, 764:1024]  # right edge
        cst[:, 3, i, :256] = AT128[0:128, 0:256]      # N=128 single window
    return cst.astype(np.float32)


# ---------------------------------------------------------------- bass module

_NC_CACHE = []


def _build_nc():
    nc = bacc.Bacc(None, target_bir_lowering=False)

    ins = {
        name: nc.dram_tensor(name, [n, n], F32, kind="ExternalInput")
        for name, n in IN_SHAPES.items()
    }
    cst_d = nc.dram_tensor("cst", [128, 4, 2, CST_W], F32, kind="ExternalInput")
    out_d = nc.dram_tensor("out", [1024, 1024], F32, kind="ExternalOutput")

    with tile.TileContext(nc) as tc:
        with tc.tile_pool(name="data", bufs=1) as dpool, \
             tc.tile_pool(name="psum", bufs=8, space="PSUM") as ppool:

            cst = dpool.tile([128, 4, 2, CST_W], F32, tag="cst", name="cst_sb")
            nc.sync.dma_start(cst, cst_d[:])

            sb = {}
            for name, n in IN_SHAPES.items():
                njb = n // 128
                t = dpool.tile([128, njb, n], F32, tag=f"in_{name}",
                               name=f"{name}_sb")
                nc.sync.dma_start(t, ins[name][:].rearrange(
                    "(b p) f -> p b f", p=128))
                sb[name] = t

            copy_ctr = [0]

            def copy_op(dst_ap, src_ap):
                # alternate DVE / ACT so psum evacuation uses both engines
                if copy_ctr[0] % 2 == 0:
                    nc.vector.tensor_copy(dst_ap, src_ap)
                else:
                    nc.scalar.copy(dst_ap, src_ap)
                copy_ctr[0] += 1

            def syn_pass(get_a, get_d, n_ob, wins, dst):
                """One 1D synthesis pass.  dst: [128, n_ob, 2N] sbuf tile.
                get_a/get_d(Wb, ob) -> [128, 128] lhsT slice (contraction block
                Wb, output-partition chunk ob)."""
                for ob in range(n_ob):
                    for Wb, (lo, hi, blk) in enumerate(wins):
                        w = hi - lo
                        ps = ppool.tile([128, CST_W], F32, tag="ps", name="ps")
                        nc.tensor.matmul(ps[:, :w], get_a(Wb, ob),
                                         cst[:, blk, 0, :w],
                                         start=True, stop=False)
                        nc.tensor.matmul(ps[:, :w], get_d(Wb, ob),
                                         cst[:, blk, 1, :w],
                                         start=False, stop=True)
                        if Wb == 0:
                            copy_op(dst[:, ob, lo:hi], ps[:, :w])
                        else:
                            # overlap strip [lo, lo+8) accumulates over windows
                            copy_op(dst[:, ob, lo + 8:hi], ps[:, 8:w])
                            nc.vector.tensor_add(dst[:, ob, lo:lo + 8],
                                                 dst[:, ob, lo:lo + 8],
                                                 ps[:, :8])

            cur = sb['ss']  # [128, 1, 128]
            for lvl, N in LEVELS:
                njb = N // 128
                wins = WINS_BY_N[N]
                a_t, sd_t = cur, sb[f'sd{lvl}']
                ds_t, dd_t = sb[f'ds{lvl}'], sb[f'dd{lvl}']

                sT = dpool.tile([128, njb, 2 * N], F32, tag=f"sT{lvl}",
                                name=f"sT{lvl}")
                dT = dpool.tile([128, njb, 2 * N], F32, tag=f"dT{lvl}",
                                name=f"dT{lvl}")
                # step 1: synthesize along H (contraction over row blocks)
                syn_pass(lambda Wb, ob: a_t[:, Wb, ob * 128:(ob + 1) * 128],
                         lambda Wb, ob: sd_t[:, Wb, ob * 128:(ob + 1) * 128],
                         njb, wins, sT)
                syn_pass(lambda Wb, ob: ds_t[:, Wb, ob * 128:(ob + 1) * 128],
                         lambda Wb, ob: dd_t[:, Wb, ob * 128:(ob + 1) * 128],
                         njb, wins, dT)
                # step 2: synthesize along W (contraction over column blocks)
                out_t = dpool.tile([128, 2 * njb, 2 * N], F32, tag=f"o{lvl}",
                                   name=f"o{lvl}")
                syn_pass(lambda Wb, ob: sT[:, Wb, ob * 128:(ob + 1) * 128],
                         lambda Wb, ob: dT[:, Wb, ob * 128:(ob + 1) * 128],
                         2 * njb, wins, out_t)
                cur = out_t

            outr = out_d[:].rearrange("(b p) f -> p b f", p=128)
            for rb in range(8):
                nc.sync.dma_start(outr[:, rb, :], cur[:, rb, :])

    nc.finalize()
    return nc


# ---------------------------------------------------------------- entry point

last_result = None  # BassKernelResults of the most recent run (for profiling)


def kernel(ss, sd0, sd1, sd2, ds0, ds1, ds2, dd0, dd1, dd2, h, g):
    b = ss.shape[0]
    assert b == 8, f"expected batch 8, got {b}"

    cst = _build_cst(np.asarray(h, np.float64), np.asarray(g, np.float64))

    full = {'ss': ss, 'sd0': sd0, 'sd1': sd1, 'sd2': sd2,
            'ds0': ds0, 'ds1': ds1, 'ds2': ds2,
            'dd0': dd0, 'dd1': dd1, 'dd2': dd2}
    in_maps = []
    for i in range(b):
        m = {name: np.ascontiguousarray(np.asarray(arr[i, 0], np.float32))
             for name, arr in full.items()}
        m['cst'] = cst
        in_maps.append(m)

    if not _NC_CACHE:
        _NC_CACHE.append(_build_nc())
    nc = _NC_CACHE[0]

    global last_result
    last_result = run_bass_kernel_spmd(nc, in_maps, core_ids=list(range(b)))
    out = np.stack([r['out'] for r in last_result.results])[:, None]
    return np.ascontiguousarray(out.astype(np.float32))
